# revision 7
# baseline (speedup 1.0000x reference)
"""Pipelined variant: V projection + A@V folded into the exp-paced pair loop:
- softmax reciprocal via single-op DVE approximation (was 6.4us InstReciprocal
  on the critical path, causing a per-pair PE stall + HAM re-throttle)
- wq/wk pre-tiled DRAM layout (2KB DMA descriptors instead of 256B)
- xt DMA split into 512-col chunks (earlier first matmul)
"""

import os
import sys

sys.path.insert(0, "/opt/trn_rl_repo")

import numpy as np

B, S, D, H = 4, 2048, 1024, 16
HD = D // H
SQ = S // 2
NCORES = 8
EPS = 1e-12

_CACHE = {}


def _install_ntff_hook():
    import contextlib
    import ctypes
    import types

    so_path = "/opt/axon/libaxon_pjrt.so"
    if "antenv.axon_hooks" in sys.modules:
        return
    try:
        lib = ctypes.CDLL(so_path)
    except OSError:
        return
    if not hasattr(lib, "axon_start_nrt_profile"):
        return
    lib.axon_start_nrt_profile.argtypes = [ctypes.POINTER(ctypes.c_int64), ctypes.c_size_t]
    lib.axon_start_nrt_profile.restype = ctypes.c_int64
    lib.axon_stop_nrt_profile.argtypes = [ctypes.c_char_p]
    lib.axon_stop_nrt_profile.restype = ctypes.c_int64

    @contextlib.contextmanager
    def _hook(output_dir, device_ids):
        import jax

        jax.devices()
        if device_ids:
            ids = (ctypes.c_int64 * len(device_ids))(*device_ids)
            rc = lib.axon_start_nrt_profile(ids, len(device_ids))
        else:
            rc = lib.axon_start_nrt_profile(None, 0)
        if rc != 0:
            raise RuntimeError(f"axon_start_nrt_profile rc={rc}")
        try:
            yield
        finally:
            n = lib.axon_stop_nrt_profile(str(output_dir).encode())
            if n < 0:
                raise RuntimeError(f"axon_stop_nrt_profile rc={n}")

    m = types.ModuleType("antenv.axon_hooks")
    m.get_axon_ntff_profile_hook = lambda: _hook
    m.set_axon_ntff_profile_hook = lambda h: None
    sys.modules["antenv.axon_hooks"] = m


def _build():
    import concourse.bass as bass
    import concourse.tile as tile
    from concourse import bacc, mybir

    f32 = mybir.dt.float32
    bf16 = mybir.dt.bfloat16
    ADD = mybir.AluOpType.add
    MULT = mybir.AluOpType.mult
    SUB = mybir.AluOpType.subtract
    Exp = mybir.ActivationFunctionType.Exp
    Sqrt = mybir.ActivationFunctionType.Sqrt

    nc = bacc.Bacc("TRN2")

    xt_d = nc.dram_tensor("xt", [D, S], bf16, kind="ExternalInput")
    xq_d = nc.dram_tensor("xq", [SQ, D], f32, kind="ExternalInput")
    wq_d = nc.dram_tensor("wqt", [128, 8, 8, 128], bf16, kind="ExternalInput")
    wk_d = nc.dram_tensor("wkt", [128, 8, 8, 128], bf16, kind="ExternalInput")
    wv_d = nc.dram_tensor("wvt", [D, D], bf16, kind="ExternalInput")
    wo_d = nc.dram_tensor("wot", [D, D], bf16, kind="ExternalInput")
    bq_d = nc.dram_tensor("bqt", [128, 8], f32, kind="ExternalInput")
    bk_d = nc.dram_tensor("bkt", [128, 8], f32, kind="ExternalInput")
    bv_d = nc.dram_tensor("bv", [D], f32, kind="ExternalInput")
    bo_d = nc.dram_tensor("bo", [D], f32, kind="ExternalInput")
    gamma_d = nc.dram_tensor("gamma", [D], f32, kind="ExternalInput")
    beta_d = nc.dram_tensor("beta", [D], f32, kind="ExternalInput")
    sel2_d = nc.dram_tensor("sel2", [2, 128], bf16, kind="ExternalInput")
    out_d = nc.dram_tensor("out", [SQ, D], f32, kind="ExternalOutput")

    def bcast_ap(handle):
        ap = handle[:]
        return bass.AP(tensor=ap.tensor, offset=ap.offset, ap=[[0, 128], ap.ap[0]])

    with tile.TileContext(nc) as tc:
        with (
            tc.tile_pool(name="const", bufs=1) as constp,
            tc.tile_pool(name="v", bufs=1) as vp,
            tc.tile_pool(name="ot", bufs=1) as otp,
            tc.tile_pool(name="xt", bufs=1) as xtp,
        ):
            bq_c = constp.tile([128, 8], f32, tag="bq")
            bk_c = constp.tile([128, 8], f32, tag="bk")
            bv_c = constp.tile([128, D], f32, tag="bv")
            bo_c = constp.tile([128, D], f32, tag="bo")
            gamma_c = constp.tile([128, D], f32, tag="gamma")
            beta_c = constp.tile([128, D], f32, tag="beta")
            eps_c = constp.tile([128, 1], f32, tag="eps")
            sel2_c = constp.tile([2, 128], bf16, tag="sel2")
            nc.sync.dma_start(out=bq_c[:], in_=bq_d[:])
            nc.sync.dma_start(out=bk_c[:], in_=bk_d[:])
            nc.gpsimd.dma_start(out=bv_c[:], in_=bcast_ap(bv_d))
            nc.gpsimd.dma_start(out=bo_c[:], in_=bcast_ap(bo_d))
            nc.gpsimd.dma_start(out=gamma_c[:], in_=bcast_ap(gamma_d))
            nc.gpsimd.dma_start(out=beta_c[:], in_=bcast_ap(beta_d))
            nc.sync.dma_start(out=sel2_c[:], in_=sel2_d[:])
            nc.vector.memset(eps_c[:], EPS)

            v = vp.tile([128, 16, H, HD + 1], bf16, tag="v")
            ot = otp.tile([128, 8, SQ], bf16, tag="ot")
            xt = xtp.tile([128, 8, S], bf16, tag="xt")

            nc.vector.memset(v[:, :, :, HD : HD + 1], 1.0)
            for cc in range(4):
                for r in range(8):
                    nc.sync.dma_start(
                        out=xt[:, r, cc * 512 : (cc + 1) * 512],
                        in_=xt_d[r * 128 : (r + 1) * 128, cc * 512 : (cc + 1) * 512],
                    )

            with (
                tc.tile_pool(name="wvr", bufs=1) as wvrp,
                tc.tile_pool(name="qkw", bufs=2) as qkwp,
                tc.tile_pool(name="qts", bufs=2) as qtsp,
                tc.tile_pool(name="kts", bufs=2) as ktsp,
                tc.tile_pool(name="st", bufs=7) as stp,
                tc.tile_pool(name="stage", bufs=2) as stagep,
                tc.tile_pool(name="dens", bufs=1) as densp,
                tc.tile_pool(name="ps1", bufs=2, space="PSUM") as ps1,
                tc.tile_pool(name="sp", bufs=2, space="PSUM") as spp,
                tc.tile_pool(name="av", bufs=1, space="PSUM") as avp,
            ):
                wv_r = wvrp.tile([128, 8, D], bf16, tag="wvr")
                for k in range(8):
                    nc.sync.dma_start(
                        out=wv_r[:, k, :], in_=wv_d[k * 128 : (k + 1) * 128, :]
                    )

                def v_chunk(lo, hi):
                    for tc_i in range(lo, hi):
                        for dg in range(2):
                            psv = ps1.tile([128, 512], f32, tag="ps", name="psv")
                            for k in range(8):
                                nc.tensor.matmul(
                                    out=psv[:],
                                    lhsT=xt[:, k, tc_i * 128 : (tc_i + 1) * 128],
                                    rhs=wv_r[:, k, dg * 512 : (dg + 1) * 512],
                                    start=(k == 0),
                                    stop=(k == 7),
                                )
                            nc.vector.tensor_tensor(
                                out=v[:, tc_i, dg * 8 : (dg + 1) * 8, 0:HD],
                                in0=psv[:].rearrange("p (h d) -> p h d", d=HD),
                                in1=bv_c[:, dg * 512 : (dg + 1) * 512].rearrange(
                                    "p (h d) -> p h d", d=HD
                                ),
                                op=ADD,
                            )

                def proj_block(m):
                    wq_m = qkwp.tile([128, 8, 128], bf16, tag="qkw", name="wq_m")
                    nc.sync.dma_start(out=wq_m[:], in_=wq_d[:, m, :, :])
                    qt_m = qtsp.tile([128, SQ], bf16, tag="qts", name="qt_m")
                    for tg in range(2):
                        psq = ps1.tile([128, 512], f32, tag="ps", name="psq")
                        for k in range(8):
                            nc.tensor.matmul(
                                out=psq[:],
                                lhsT=wq_m[:, k, :],
                                rhs=xt[:, k, tg * 512 : (tg + 1) * 512],
                                start=(k == 0),
                                stop=(k == 7),
                            )
                        nc.vector.tensor_scalar(
                            out=qt_m[:, tg * 512 : (tg + 1) * 512],
                            in0=psq[:],
                            scalar1=bq_c[:, m : m + 1],
                            scalar2=None,
                            op0=ADD,
                        )
                    wk_m = qkwp.tile([128, 8, 128], bf16, tag="qkw", name="wk_m")
                    nc.sync.dma_start(out=wk_m[:], in_=wk_d[:, m, :, :])
                    kt_m = ktsp.tile([128, S], bf16, tag="kts", name="kt_m")
                    for tg in range(4):
                        psk = ps1.tile([128, 512], f32, tag="ps", name="psk")
                        for k in range(8):
                            nc.tensor.matmul(
                                out=psk[:],
                                lhsT=wk_m[:, k, :],
                                rhs=xt[:, k, tg * 512 : (tg + 1) * 512],
                                start=(k == 0),
                                stop=(k == 7),
                            )
                        nc.vector.tensor_scalar(
                            out=kt_m[:, tg * 512 : (tg + 1) * 512],
                            in0=psk[:],
                            scalar1=bk_c[:, m : m + 1],
                            scalar2=None,
                            op0=ADD,
                        )
                    return qt_m, kt_m

                def av_tile_block(m, hh, j, av, st_j):
                    # A@V for key-chunks 4j..4j+3 of head 2m+hh, both query
                    # halves, so st tile j is fully consumed (early release).
                    h = 2 * m + hh
                    for qh in range(2):
                        for kc in range(4 * j, 4 * j + 4):
                            nc.tensor.matmul(
                                out=av[0:65, qh * 512 : (qh + 1) * 512],
                                lhsT=v[:, kc, h, :],
                                rhs=st_j[:, kc % 4, qh * 512 : (qh + 1) * 512],
                                start=(kc == 0),
                                stop=(kc == 15),
                                skip_group_check=True,
                            )

                def head_evac(m, hh, av, den_m):
                    stg_d = stagep.tile([65, SQ], f32, tag="stg_d", name="stg_d")
                    nc.vector.tensor_copy(out=stg_d[64:65, :], in_=av[64:65, :])
                    nc.sync.dma_start(out=den_m[hh : hh + 1, :], in_=stg_d[64:65, :])
                    if hh == 0:
                        nc.vector.tensor_copy(out=ot[0:64, m, :], in_=av[0:64, :])
                    else:
                        stg_o = stagep.tile([64, SQ], bf16, tag="stg_o", name="stg_o")
                        nc.vector.tensor_copy(out=stg_o[:, :], in_=av[0:64, :])
                        nc.sync.dma_start(out=ot[64:128, m, :], in_=stg_o[:, :])

                def av_finish(m, den_m):
                    rc_f = densp.tile([2, SQ], f32, tag="rcf", name="rc_f")
                    rc_b = densp.tile([2, SQ], bf16, tag="rcb", name="rc_b")
                    nc.vector.reciprocal_approx_fast(out=rc_f[:], in_=den_m[:])
                    nc.vector.tensor_copy(out=rc_b[:], in_=rc_f[:])
                    bc = avp.tile([128, 1024], f32, tag="av", name="bc")
                    for g in range(2):
                        nc.tensor.matmul(
                            out=bc[:, g * 512 : (g + 1) * 512],
                            lhsT=sel2_c[:],
                            rhs=rc_b[:, g * 512 : (g + 1) * 512],
                            start=True,
                            stop=True,
                        )
                    nc.vector.tensor_tensor(out=ot[:, m, :], in0=ot[:, m, :], in1=bc[:], op=MULT)

                qt_m, kt_m = proj_block(0)
                nxt = None
                for m in range(8):
                    st_pair = [
                        [
                            stp.tile([128, 4, SQ], bf16, tag="st", name="st")
                            for _ in range(4)
                        ]
                        for _ in range(2)
                    ]
                    den_m = densp.tile([2, SQ], f32, tag="den", name="den_m")
                    for hh in range(2):
                        p0 = hh * 64
                        av = avp.tile([128, 1024], f32, tag="av", name="av")
                        for kc in range(16):
                            sp = spp.tile([128, 1024], f32, tag="sp", name="sp")
                            for qh in range(2):
                                nc.tensor.matmul(
                                    out=sp[:, qh * 512 : (qh + 1) * 512],
                                    lhsT=kt_m[p0 : p0 + 64, kc * 128 : (kc + 1) * 128],
                                    rhs=qt_m[p0 : p0 + 64, qh * 512 : (qh + 1) * 512],
                                    start=True,
                                    stop=True,
                                )
                            nc.scalar.activation(
                                out=st_pair[hh][kc // 4][:, kc % 4, :],
                                in_=sp[:],
                                func=Exp,
                                scale=float(1.0 / np.sqrt(HD)),
                            )
                            if hh == 0 and kc == 3 and m < 7:
                                nxt = proj_block(m + 1)
                            if kc % 4 == 3:
                                j = kc // 4
                                if m == 0 and hh == 0:
                                    v_chunk(4 * j, 4 * j + 4)
                                av_tile_block(m, hh, j, av, st_pair[hh][j])
                        head_evac(m, hh, av, den_m)
                    av_finish(m, den_m)
                    if nxt is not None:
                        qt_m, kt_m = nxt
                        nxt = None

            with (
                tc.tile_pool(name="wo", bufs=8) as wop,
                tc.tile_pool(name="xqp", bufs=8) as xqp,
                tc.tile_pool(name="xqbo", bufs=8) as xqbop,
                tc.tile_pool(name="y", bufs=3) as yp,
                tc.tile_pool(name="y2", bufs=3) as y2p,
                tc.tile_pool(name="stats", bufs=4) as statp,
                tc.tile_pool(name="ps3", bufs=4, space="PSUM") as ps3,
            ):
                wo_tiles = []
                for k in range(8):
                    wt = wop.tile([128, D], bf16, tag="wo", name="wo_t")
                    nc.sync.dma_start(out=wt[:], in_=wo_d[k * 128 : (k + 1) * 128, :])
                    wo_tiles.append(wt)
                xq_tiles = []
                for t in range(8):
                    xq_t = xqp.tile([128, D], f32, tag="xq", name="xq_t")
                    nc.sync.dma_start(out=xq_t[:], in_=xq_d[t * 128 : (t + 1) * 128, :])
                    xq_tiles.append(xq_t)
                xqbo_tiles = []
                for t in range(8):
                    xqbo = xqbop.tile([128, D], f32, tag="xqbo", name="xqbo_t")
                    nc.gpsimd.tensor_tensor(
                        out=xqbo[:], in0=xq_tiles[t][:], in1=bo_c[:], op=ADD
                    )
                    xqbo_tiles.append(xqbo)
                for tg in range(2):
                    ps = [ps3.tile([128, 1024], f32, tag="ps", name="ps3") for _ in range(4)]
                    for k in range(8):
                        wt = wo_tiles[k]
                        for i in range(4):
                            t = tg * 4 + i
                            for g in range(2):
                                nc.tensor.matmul(
                                    out=ps[i][:, g * 512 : (g + 1) * 512],
                                    lhsT=ot[:, k, t * 128 : (t + 1) * 128],
                                    rhs=wt[:, g * 512 : (g + 1) * 512],
                                    start=(k == 0),
                                    stop=(k == 7),
                                )
                    for i in range(4):
                        t = tg * 4 + i
                        xqbo = xqbo_tiles[t]
                        y = yp.tile([128, D], f32, tag="y")
                        nc.vector.tensor_tensor(out=y[:], in0=ps[i][:], in1=xqbo[:], op=ADD)
                        stats = statp.tile([128, 2, 6], f32, tag="stats")
                        mv = statp.tile([128, 2], f32, tag="mv")
                        nc.vector.bn_stats(out=stats[:, 0, :], in_=y[:, 0:512])
                        nc.vector.bn_stats(out=stats[:, 1, :], in_=y[:, 512:1024])
                        nc.vector.bn_aggr(out=mv[:], in_=stats[:])
                        nc.scalar.activation(
                            out=mv[:, 1:2], in_=mv[:, 1:2], func=Sqrt, bias=eps_c[:, 0:1]
                        )
                        nc.vector.reciprocal(out=mv[:, 1:2], in_=mv[:, 1:2])
                        nc.vector.tensor_scalar(
                            out=y[:],
                            in0=y[:],
                            scalar1=mv[:, 0:1],
                            scalar2=mv[:, 1:2],
                            op0=SUB,
                            op1=MULT,
                        )
                        y2 = y2p.tile([128, D], f32, tag="y2")
                        nc.vector.tensor_tensor(out=y2[:], in0=y[:], in1=gamma_c[:], op=MULT)
                        nc.gpsimd.tensor_tensor(out=y2[:], in0=y2[:], in1=beta_c[:], op=ADD)
                        nc.sync.dma_start(out=out_d[t * 128 : (t + 1) * 128, :], in_=y2[:])

    nc.compile()
    return nc


def _get_nc():
    if "nc" not in _CACHE:
        _CACHE["nc"] = _build()
    return _CACHE["nc"]


def kernel(X, Wq, bq, Wk, bk, Wv, bv, Wo, bo, gamma, beta):
    if os.environ.get("BASS_TRACE"):
        _install_ntff_hook()
    import ml_dtypes

    from concourse.bass_utils import run_bass_kernel_spmd

    bfdt = ml_dtypes.bfloat16
    f32 = np.float32
    X = np.ascontiguousarray(np.asarray(X, dtype=f32))
    wqt = np.ascontiguousarray(
        np.asarray(Wq, f32).T.astype(bfdt).reshape(8, 128, 8, 128).transpose(1, 2, 0, 3)
    )
    wkt = np.ascontiguousarray(
        np.asarray(Wk, f32).T.astype(bfdt).reshape(8, 128, 8, 128).transpose(1, 2, 0, 3)
    )
    wvt = np.ascontiguousarray(np.asarray(Wv, f32).T.astype(bfdt))
    wot = np.ascontiguousarray(np.asarray(Wo, f32).T.astype(bfdt))
    bqt = np.ascontiguousarray(np.asarray(bq, f32).reshape(8, 128).T)
    bkt = np.ascontiguousarray(np.asarray(bk, f32).reshape(8, 128).T)
    bv_ = np.ascontiguousarray(np.asarray(bv, f32))
    bo_ = np.ascontiguousarray(np.asarray(bo, f32))
    gamma_ = np.ascontiguousarray(np.asarray(gamma, f32))
    beta_ = np.ascontiguousarray(np.asarray(beta, f32))
    sel2 = np.zeros((2, 128), f32)
    sel2[0, 0:64] = 1.0
    sel2[1, 64:128] = 1.0
    sel2 = sel2.astype(bfdt)

    in_maps = []
    for c in range(NCORES):
        b, half = c // 2, c % 2
        Xb = X[b]
        q_rows = Xb[half * SQ : (half + 1) * SQ]
        o_rows = Xb[(1 - half) * SQ : (2 - half) * SQ]
        xt = np.ascontiguousarray(np.concatenate([q_rows, o_rows], axis=0).T.astype(bfdt))
        in_maps.append(
            {
                "xt": xt,
                "xq": np.ascontiguousarray(q_rows),
                "wqt": wqt,
                "wkt": wkt,
                "wvt": wvt,
                "wot": wot,
                "bqt": bqt,
                "bkt": bkt,
                "bv": bv_,
                "bo": bo_,
                "gamma": gamma_,
                "beta": beta_,
                "sel2": sel2,
            }
        )

    nc = _get_nc()
    res = run_bass_kernel_spmd(nc, in_maps, core_ids=list(range(NCORES)))
    if res.exec_time_ns is not None:
        print(f"HW exec time: {res.exec_time_ns} ns")

    out = np.empty((B, S, D), np.float32)
    for c in range(NCORES):
        b, half = c // 2, c % 2
        out[b, half * SQ : (half + 1) * SQ] = res.results[c]["out"]
    return out


# revision 9
# speedup vs baseline: 1.0494x; 1.0494x over previous
"""Baseline structure + targeted fixes only:
- softmax reciprocal via single-op DVE approximation (was 6.4us InstReciprocal
  on the critical path, causing a per-pair PE stall + HAM re-throttle)
- wq/wk pre-tiled DRAM layout (2KB DMA descriptors instead of 256B)
- xt DMA split into 512-col chunks (earlier first matmul)
"""

import os
import sys

sys.path.insert(0, "/opt/trn_rl_repo")

import numpy as np

B, S, D, H = 4, 2048, 1024, 16
HD = D // H
SQ = S // 2
NCORES = 8
EPS = 1e-12

_CACHE = {}


def _install_ntff_hook():
    import contextlib
    import ctypes
    import types

    so_path = "/opt/axon/libaxon_pjrt.so"
    if "antenv.axon_hooks" in sys.modules:
        return
    try:
        lib = ctypes.CDLL(so_path)
    except OSError:
        return
    if not hasattr(lib, "axon_start_nrt_profile"):
        return
    lib.axon_start_nrt_profile.argtypes = [ctypes.POINTER(ctypes.c_int64), ctypes.c_size_t]
    lib.axon_start_nrt_profile.restype = ctypes.c_int64
    lib.axon_stop_nrt_profile.argtypes = [ctypes.c_char_p]
    lib.axon_stop_nrt_profile.restype = ctypes.c_int64

    @contextlib.contextmanager
    def _hook(output_dir, device_ids):
        import jax

        jax.devices()
        if device_ids:
            ids = (ctypes.c_int64 * len(device_ids))(*device_ids)
            rc = lib.axon_start_nrt_profile(ids, len(device_ids))
        else:
            rc = lib.axon_start_nrt_profile(None, 0)
        if rc != 0:
            raise RuntimeError(f"axon_start_nrt_profile rc={rc}")
        try:
            yield
        finally:
            n = lib.axon_stop_nrt_profile(str(output_dir).encode())
            if n < 0:
                raise RuntimeError(f"axon_stop_nrt_profile rc={n}")

    m = types.ModuleType("antenv.axon_hooks")
    m.get_axon_ntff_profile_hook = lambda: _hook
    m.set_axon_ntff_profile_hook = lambda h: None
    sys.modules["antenv.axon_hooks"] = m


def _build():
    import concourse.bass as bass
    import concourse.tile as tile
    from concourse import bacc, mybir

    f32 = mybir.dt.float32
    bf16 = mybir.dt.bfloat16
    ADD = mybir.AluOpType.add
    MULT = mybir.AluOpType.mult
    SUB = mybir.AluOpType.subtract
    Exp = mybir.ActivationFunctionType.Exp
    Sqrt = mybir.ActivationFunctionType.Sqrt

    nc = bacc.Bacc("TRN2")

    xt_d = nc.dram_tensor("xt", [D, S], bf16, kind="ExternalInput")
    xq_d = nc.dram_tensor("xq", [SQ, D], f32, kind="ExternalInput")
    wq_d = nc.dram_tensor("wqt", [128, 8, 8, 128], bf16, kind="ExternalInput")
    wk_d = nc.dram_tensor("wkt", [128, 8, 8, 128], bf16, kind="ExternalInput")
    wv_d = nc.dram_tensor("wvt", [D, D], bf16, kind="ExternalInput")
    wo_d = nc.dram_tensor("wot", [D, D], bf16, kind="ExternalInput")
    bq_d = nc.dram_tensor("bqt", [128, 8], f32, kind="ExternalInput")
    bk_d = nc.dram_tensor("bkt", [128, 8], f32, kind="ExternalInput")
    bv_d = nc.dram_tensor("bv", [D], f32, kind="ExternalInput")
    bo_d = nc.dram_tensor("bo", [D], f32, kind="ExternalInput")
    gamma_d = nc.dram_tensor("gamma", [D], f32, kind="ExternalInput")
    beta_d = nc.dram_tensor("beta", [D], f32, kind="ExternalInput")
    sel2_d = nc.dram_tensor("sel2", [2, 128], bf16, kind="ExternalInput")
    out_d = nc.dram_tensor("out", [SQ, D], f32, kind="ExternalOutput")

    def bcast_ap(handle):
        ap = handle[:]
        return bass.AP(tensor=ap.tensor, offset=ap.offset, ap=[[0, 128], ap.ap[0]])

    with tile.TileContext(nc) as tc:
        with (
            tc.tile_pool(name="const", bufs=1) as constp,
            tc.tile_pool(name="v", bufs=1) as vp,
            tc.tile_pool(name="ot", bufs=1) as otp,
            tc.tile_pool(name="xt", bufs=1) as xtp,
        ):
            bq_c = constp.tile([128, 8], f32, tag="bq")
            bk_c = constp.tile([128, 8], f32, tag="bk")
            bv_c = constp.tile([128, D], f32, tag="bv")
            bo_c = constp.tile([128, D], f32, tag="bo")
            gamma_c = constp.tile([128, D], f32, tag="gamma")
            beta_c = constp.tile([128, D], f32, tag="beta")
            eps_c = constp.tile([128, 1], f32, tag="eps")
            sel2_c = constp.tile([2, 128], bf16, tag="sel2")
            nc.vector.memset(eps_c[:], EPS)

            v = vp.tile([128, 16, H, HD + 1], bf16, tag="v")
            ot = otp.tile([128, 8, SQ], bf16, tag="ot")
            xt = xtp.tile([128, 8, S], bf16, tag="xt")

            nc.vector.memset(v[:, :, :, HD : HD + 1], 1.0)

            with (
                tc.tile_pool(name="wvr", bufs=1) as wvrp,
                tc.tile_pool(name="qkw", bufs=2) as qkwp,
                tc.tile_pool(name="qts", bufs=2) as qtsp,
                tc.tile_pool(name="kts", bufs=2) as ktsp,
                tc.tile_pool(name="st", bufs=6) as stp,
                tc.tile_pool(name="stage", bufs=2) as stagep,
                tc.tile_pool(name="dens", bufs=1) as densp,
                tc.tile_pool(name="ps1", bufs=2, space="PSUM") as ps1,
                tc.tile_pool(name="sp", bufs=2, space="PSUM") as spp,
                tc.tile_pool(name="av", bufs=1, space="PSUM") as avp,
            ):
                wv_r = wvrp.tile([128, 8, D], bf16, tag="wvr")
                for k in range(8):
                    nc.sync.dma_start(
                        out=wv_r[:, k, :], in_=wv_d[k * 128 : (k + 1) * 128, :]
                    )
                for r in range(8):
                    nc.sync.dma_start(
                        out=xt[:, r, 0:512], in_=xt_d[r * 128 : (r + 1) * 128, 0:512]
                    )
                for cc in range(1, 4):
                    for r in range(8):
                        nc.gpsimd.dma_start(
                            out=xt[:, r, cc * 512 : (cc + 1) * 512],
                            in_=xt_d[r * 128 : (r + 1) * 128, cc * 512 : (cc + 1) * 512],
                        )
                nc.sync.dma_start(out=bq_c[:], in_=bq_d[:])
                nc.sync.dma_start(out=bk_c[:], in_=bk_d[:])
                nc.sync.dma_start(out=sel2_c[:], in_=sel2_d[:])
                nc.gpsimd.dma_start(out=bv_c[:], in_=bcast_ap(bv_d))
                nc.gpsimd.dma_start(out=bo_c[:], in_=bcast_ap(bo_d))
                nc.gpsimd.dma_start(out=gamma_c[:], in_=bcast_ap(gamma_d))
                nc.gpsimd.dma_start(out=beta_c[:], in_=bcast_ap(beta_d))
                for tc_i in range(16):
                    for dg in range(2):
                        psv = ps1.tile([128, 512], f32, tag="ps", name="psv")
                        for k in range(8):
                            nc.tensor.matmul(
                                out=psv[:],
                                lhsT=xt[:, k, tc_i * 128 : (tc_i + 1) * 128],
                                rhs=wv_r[:, k, dg * 512 : (dg + 1) * 512],
                                start=(k == 0),
                                stop=(k == 7),
                            )
                        nc.vector.tensor_tensor(
                            out=v[:, tc_i, dg * 8 : (dg + 1) * 8, 0:HD],
                            in0=psv[:].rearrange("p (h d) -> p h d", d=HD),
                            in1=bv_c[:, dg * 512 : (dg + 1) * 512].rearrange(
                                "p (h d) -> p h d", d=HD
                            ),
                            op=ADD,
                        )

                def proj_block(m):
                    wq_m = qkwp.tile([128, 8, 128], bf16, tag="qkw", name="wq_m")
                    nc.sync.dma_start(out=wq_m[:], in_=wq_d[:, m, :, :])
                    qt_m = qtsp.tile([128, SQ], bf16, tag="qts", name="qt_m")
                    for tg in range(2):
                        psq = ps1.tile([128, 512], f32, tag="ps", name="psq")
                        for k in range(8):
                            nc.tensor.matmul(
                                out=psq[:],
                                lhsT=wq_m[:, k, :],
                                rhs=xt[:, k, tg * 512 : (tg + 1) * 512],
                                start=(k == 0),
                                stop=(k == 7),
                            )
                        nc.vector.tensor_scalar(
                            out=qt_m[:, tg * 512 : (tg + 1) * 512],
                            in0=psq[:],
                            scalar1=bq_c[:, m : m + 1],
                            scalar2=None,
                            op0=ADD,
                        )
                    wk_m = qkwp.tile([128, 8, 128], bf16, tag="qkw", name="wk_m")
                    nc.sync.dma_start(out=wk_m[:], in_=wk_d[:, m, :, :])
                    kt_m = ktsp.tile([128, S], bf16, tag="kts", name="kt_m")
                    for tg in range(4):
                        psk = ps1.tile([128, 512], f32, tag="ps", name="psk")
                        for k in range(8):
                            nc.tensor.matmul(
                                out=psk[:],
                                lhsT=wk_m[:, k, :],
                                rhs=xt[:, k, tg * 512 : (tg + 1) * 512],
                                start=(k == 0),
                                stop=(k == 7),
                            )
                        nc.vector.tensor_scalar(
                            out=kt_m[:, tg * 512 : (tg + 1) * 512],
                            in0=psk[:],
                            scalar1=bk_c[:, m : m + 1],
                            scalar2=None,
                            op0=ADD,
                        )
                    return qt_m, kt_m

                def qk_exp_block(m, qt_m, kt_m):
                    st_pair = []
                    for hh in range(2):
                        p0 = hh * 64
                        st_tiles = [
                            stp.tile([128, 4, SQ], bf16, tag="st", name="st")
                            for _ in range(4)
                        ]
                        for kc in range(16):
                            sp = spp.tile([128, 1024], f32, tag="sp", name="sp")
                            for qh in range(2):
                                nc.tensor.matmul(
                                    out=sp[:, qh * 512 : (qh + 1) * 512],
                                    lhsT=kt_m[p0 : p0 + 64, kc * 128 : (kc + 1) * 128],
                                    rhs=qt_m[p0 : p0 + 64, qh * 512 : (qh + 1) * 512],
                                    start=True,
                                    stop=True,
                                )
                            nc.scalar.activation(
                                out=st_tiles[kc // 4][:, kc % 4, :],
                                in_=sp[:],
                                func=Exp,
                                scale=float(1.0 / np.sqrt(HD)),
                            )
                        st_pair.append(st_tiles)
                    return st_pair

                def av_block(m, st_pair):
                    den_m = densp.tile([2, SQ], f32, tag="den", name="den_m")
                    for hh in range(2):
                        h = 2 * m + hh
                        st_tiles = st_pair[hh]
                        av = avp.tile([128, 1024], f32, tag="av", name="av")
                        for qh in range(2):
                            for kc in range(16):
                                nc.tensor.matmul(
                                    out=av[0:65, qh * 512 : (qh + 1) * 512],
                                    lhsT=v[:, kc, h, :],
                                    rhs=st_tiles[kc // 4][
                                        :, kc % 4, qh * 512 : (qh + 1) * 512
                                    ],
                                    start=(kc == 0),
                                    stop=(kc == 15),
                                )
                        stg_d = stagep.tile([65, SQ], f32, tag="stg_d", name="stg_d")
                        nc.vector.tensor_copy(out=stg_d[64:65, :], in_=av[64:65, :])
                        nc.sync.dma_start(out=den_m[hh : hh + 1, :], in_=stg_d[64:65, :])
                        if hh == 0:
                            nc.vector.tensor_copy(out=ot[0:64, m, :], in_=av[0:64, :])
                        else:
                            stg_o = stagep.tile([64, SQ], bf16, tag="stg_o", name="stg_o")
                            nc.vector.tensor_copy(out=stg_o[:, :], in_=av[0:64, :])
                            nc.sync.dma_start(out=ot[64:128, m, :], in_=stg_o[:, :])
                    rc_f = densp.tile([2, SQ], f32, tag="rcf", name="rc_f")
                    rc_b = densp.tile([2, SQ], bf16, tag="rcb", name="rc_b")
                    nc.vector.reciprocal_approx_fast(out=rc_f[:], in_=den_m[:])
                    nc.vector.tensor_copy(out=rc_b[:], in_=rc_f[:])
                    bc = avp.tile([128, 1024], f32, tag="av", name="bc")
                    for g in range(2):
                        nc.tensor.matmul(
                            out=bc[:, g * 512 : (g + 1) * 512],
                            lhsT=sel2_c[:],
                            rhs=rc_b[:, g * 512 : (g + 1) * 512],
                            start=True,
                            stop=True,
                        )
                    nc.vector.tensor_tensor(out=ot[:, m, :], in0=ot[:, m, :], in1=bc[:], op=MULT)

                prev = None
                for m in range(8):
                    qt_m, kt_m = proj_block(m)
                    st_pair = qk_exp_block(m, qt_m, kt_m)
                    if prev is not None:
                        av_block(prev[0], prev[1])
                    prev = (m, st_pair)
                av_block(prev[0], prev[1])

            with (
                tc.tile_pool(name="wo", bufs=8) as wop,
                tc.tile_pool(name="xqp", bufs=8) as xqp,
                tc.tile_pool(name="xqbo", bufs=8) as xqbop,
                tc.tile_pool(name="y", bufs=3) as yp,
                tc.tile_pool(name="y2", bufs=3) as y2p,
                tc.tile_pool(name="stats", bufs=4) as statp,
                tc.tile_pool(name="ps3", bufs=4, space="PSUM") as ps3,
            ):
                wo_tiles = []
                for k in range(8):
                    wt = wop.tile([128, D], bf16, tag="wo", name="wo_t")
                    nc.sync.dma_start(out=wt[:], in_=wo_d[k * 128 : (k + 1) * 128, :])
                    wo_tiles.append(wt)
                xq_tiles = []
                for t in range(8):
                    xq_t = xqp.tile([128, D], f32, tag="xq", name="xq_t")
                    nc.sync.dma_start(out=xq_t[:], in_=xq_d[t * 128 : (t + 1) * 128, :])
                    xq_tiles.append(xq_t)
                xqbo_tiles = []
                for t in range(8):
                    xqbo = xqbop.tile([128, D], f32, tag="xqbo", name="xqbo_t")
                    nc.gpsimd.tensor_tensor(
                        out=xqbo[:], in0=xq_tiles[t][:], in1=bo_c[:], op=ADD
                    )
                    xqbo_tiles.append(xqbo)
                for tg in range(2):
                    ps = [ps3.tile([128, 1024], f32, tag="ps", name="ps3") for _ in range(4)]
                    for k in range(8):
                        wt = wo_tiles[k]
                        for i in range(4):
                            t = tg * 4 + i
                            for g in range(2):
                                nc.tensor.matmul(
                                    out=ps[i][:, g * 512 : (g + 1) * 512],
                                    lhsT=ot[:, k, t * 128 : (t + 1) * 128],
                                    rhs=wt[:, g * 512 : (g + 1) * 512],
                                    start=(k == 0),
                                    stop=(k == 7),
                                )
                    for i in range(4):
                        t = tg * 4 + i
                        xqbo = xqbo_tiles[t]
                        y = yp.tile([128, D], f32, tag="y")
                        nc.vector.tensor_tensor(out=y[:], in0=ps[i][:], in1=xqbo[:], op=ADD)
                        stats = statp.tile([128, 2, 6], f32, tag="stats")
                        mv = statp.tile([128, 2], f32, tag="mv")
                        nc.vector.bn_stats(out=stats[:, 0, :], in_=y[:, 0:512])
                        nc.vector.bn_stats(out=stats[:, 1, :], in_=y[:, 512:1024])
                        nc.vector.bn_aggr(out=mv[:], in_=stats[:])
                        nc.scalar.activation(
                            out=mv[:, 1:2], in_=mv[:, 1:2], func=Sqrt, bias=eps_c[:, 0:1]
                        )
                        nc.vector.reciprocal(out=mv[:, 1:2], in_=mv[:, 1:2])
                        nc.vector.tensor_scalar(
                            out=y[:],
                            in0=y[:],
                            scalar1=mv[:, 0:1],
                            scalar2=mv[:, 1:2],
                            op0=SUB,
                            op1=MULT,
                        )
                        y2 = y2p.tile([128, D], f32, tag="y2")
                        nc.vector.tensor_tensor(out=y2[:], in0=y[:], in1=gamma_c[:], op=MULT)
                        nc.gpsimd.tensor_tensor(out=y2[:], in0=y2[:], in1=beta_c[:], op=ADD)
                        nc.sync.dma_start(out=out_d[t * 128 : (t + 1) * 128, :], in_=y2[:])

    nc.compile()
    return nc


def _get_nc():
    if "nc" not in _CACHE:
        _CACHE["nc"] = _build()
    return _CACHE["nc"]


def kernel(X, Wq, bq, Wk, bk, Wv, bv, Wo, bo, gamma, beta):
    if os.environ.get("BASS_TRACE"):
        _install_ntff_hook()
    import ml_dtypes

    from concourse.bass_utils import run_bass_kernel_spmd

    bfdt = ml_dtypes.bfloat16
    f32 = np.float32
    X = np.ascontiguousarray(np.asarray(X, dtype=f32))
    wqt = np.ascontiguousarray(
        np.asarray(Wq, f32).T.astype(bfdt).reshape(8, 128, 8, 128).transpose(1, 2, 0, 3)
    )
    wkt = np.ascontiguousarray(
        np.asarray(Wk, f32).T.astype(bfdt).reshape(8, 128, 8, 128).transpose(1, 2, 0, 3)
    )
    wvt = np.ascontiguousarray(np.asarray(Wv, f32).T.astype(bfdt))
    wot = np.ascontiguousarray(np.asarray(Wo, f32).T.astype(bfdt))
    bqt = np.ascontiguousarray(np.asarray(bq, f32).reshape(8, 128).T)
    bkt = np.ascontiguousarray(np.asarray(bk, f32).reshape(8, 128).T)
    bv_ = np.ascontiguousarray(np.asarray(bv, f32))
    bo_ = np.ascontiguousarray(np.asarray(bo, f32))
    gamma_ = np.ascontiguousarray(np.asarray(gamma, f32))
    beta_ = np.ascontiguousarray(np.asarray(beta, f32))
    sel2 = np.zeros((2, 128), f32)
    sel2[0, 0:64] = 1.0
    sel2[1, 64:128] = 1.0
    sel2 = sel2.astype(bfdt)

    in_maps = []
    for c in range(NCORES):
        b, half = c // 2, c % 2
        Xb = X[b]
        q_rows = Xb[half * SQ : (half + 1) * SQ]
        o_rows = Xb[(1 - half) * SQ : (2 - half) * SQ]
        xt = np.ascontiguousarray(np.concatenate([q_rows, o_rows], axis=0).T.astype(bfdt))
        in_maps.append(
            {
                "xt": xt,
                "xq": np.ascontiguousarray(q_rows),
                "wqt": wqt,
                "wkt": wkt,
                "wvt": wvt,
                "wot": wot,
                "bqt": bqt,
                "bkt": bkt,
                "bv": bv_,
                "bo": bo_,
                "gamma": gamma_,
                "beta": beta_,
                "sel2": sel2,
            }
        )

    nc = _get_nc()
    res = run_bass_kernel_spmd(nc, in_maps, core_ids=list(range(NCORES)))
    if res.exec_time_ns is not None:
        print(f"HW exec time: {res.exec_time_ns} ns")

    out = np.empty((B, S, D), np.float32)
    for c in range(NCORES):
        b, half = c // 2, c % 2
        out[b, half * SQ : (half + 1) * SQ] = res.results[c]["out"]
    return out


# revision 14
# speedup vs baseline: 1.1261x; 1.0730x over previous
"""Baseline structure + targeted fixes only:
- softmax reciprocal via single-op DVE approximation (was 6.4us InstReciprocal
  on the critical path, causing a per-pair PE stall + HAM re-throttle)
- wq/wk pre-tiled DRAM layout (2KB DMA descriptors instead of 256B)
- xt DMA split into 512-col chunks (earlier first matmul)
"""

import os
import sys

sys.path.insert(0, "/opt/trn_rl_repo")

import numpy as np

B, S, D, H = 4, 2048, 1024, 16
HD = D // H
SQ = S // 2
NCORES = 8
EPS = 1e-12

_CACHE = {}


def _install_ntff_hook():
    import contextlib
    import ctypes
    import types

    so_path = "/opt/axon/libaxon_pjrt.so"
    if "antenv.axon_hooks" in sys.modules:
        return
    try:
        lib = ctypes.CDLL(so_path)
    except OSError:
        return
    if not hasattr(lib, "axon_start_nrt_profile"):
        return
    lib.axon_start_nrt_profile.argtypes = [ctypes.POINTER(ctypes.c_int64), ctypes.c_size_t]
    lib.axon_start_nrt_profile.restype = ctypes.c_int64
    lib.axon_stop_nrt_profile.argtypes = [ctypes.c_char_p]
    lib.axon_stop_nrt_profile.restype = ctypes.c_int64

    @contextlib.contextmanager
    def _hook(output_dir, device_ids):
        import jax

        jax.devices()
        if device_ids:
            ids = (ctypes.c_int64 * len(device_ids))(*device_ids)
            rc = lib.axon_start_nrt_profile(ids, len(device_ids))
        else:
            rc = lib.axon_start_nrt_profile(None, 0)
        if rc != 0:
            raise RuntimeError(f"axon_start_nrt_profile rc={rc}")
        try:
            yield
        finally:
            n = lib.axon_stop_nrt_profile(str(output_dir).encode())
            if n < 0:
                raise RuntimeError(f"axon_stop_nrt_profile rc={n}")

    m = types.ModuleType("antenv.axon_hooks")
    m.get_axon_ntff_profile_hook = lambda: _hook
    m.set_axon_ntff_profile_hook = lambda h: None
    sys.modules["antenv.axon_hooks"] = m


def _build():
    import concourse.bass as bass
    import concourse.tile as tile
    from concourse import bacc, mybir

    f32 = mybir.dt.float32
    bf16 = mybir.dt.bfloat16
    f8 = mybir.dt.float8e4
    ADD = mybir.AluOpType.add
    MULT = mybir.AluOpType.mult
    SUB = mybir.AluOpType.subtract
    Exp = mybir.ActivationFunctionType.Exp
    Sqrt = mybir.ActivationFunctionType.Sqrt

    nc = bacc.Bacc("TRN2")

    xt_d = nc.dram_tensor("xt", [D, S], f8, kind="ExternalInput")
    xq_d = nc.dram_tensor("xq", [SQ, D], f32, kind="ExternalInput")
    wq_d = nc.dram_tensor("wqt", [128, 8, 8, 128], f8, kind="ExternalInput")
    wk_d = nc.dram_tensor("wkt", [128, 8, 8, 128], f8, kind="ExternalInput")
    wv_d = nc.dram_tensor("wvt", [D, D], f8, kind="ExternalInput")
    wo_d = nc.dram_tensor("wot", [D, D], bf16, kind="ExternalInput")
    bq_d = nc.dram_tensor("bqt", [128, 8], f32, kind="ExternalInput")
    bk_d = nc.dram_tensor("bkt", [128, 8], f32, kind="ExternalInput")
    bv_d = nc.dram_tensor("bv", [D], f32, kind="ExternalInput")
    bo_d = nc.dram_tensor("bo", [D], f32, kind="ExternalInput")
    gamma_d = nc.dram_tensor("gamma", [D], f32, kind="ExternalInput")
    beta_d = nc.dram_tensor("beta", [D], f32, kind="ExternalInput")
    sel2_d = nc.dram_tensor("sel2", [2, 128], bf16, kind="ExternalInput")
    out_d = nc.dram_tensor("out", [SQ, D], f32, kind="ExternalOutput")

    def bcast_ap(handle):
        ap = handle[:]
        return bass.AP(tensor=ap.tensor, offset=ap.offset, ap=[[0, 128], ap.ap[0]])

    with tile.TileContext(nc) as tc:
        with (
            tc.tile_pool(name="const", bufs=1) as constp,
            tc.tile_pool(name="v", bufs=1) as vp,
            tc.tile_pool(name="ot", bufs=1) as otp,
            tc.tile_pool(name="xt", bufs=1) as xtp,
        ):
            bq_c = constp.tile([128, 8], f32, tag="bq")
            bk_c = constp.tile([128, 8], f32, tag="bk")
            bv_c = constp.tile([128, D], f32, tag="bv")
            bo_c = constp.tile([128, D], f32, tag="bo")
            gamma_c = constp.tile([128, D], f32, tag="gamma")
            beta_c = constp.tile([128, D], f32, tag="beta")
            eps_c = constp.tile([128, 1], f32, tag="eps")
            negone_c = constp.tile([128, 1], f32, tag="negone")
            sel2_c = constp.tile([2, 128], bf16, tag="sel2")
            nc.vector.memset(eps_c[:], EPS)
            nc.vector.memset(negone_c[:], -3.5)

            v = vp.tile([128, 16, H, HD + 1], f8, tag="v")
            ot = otp.tile([128, 8, SQ], bf16, tag="ot")
            xt = xtp.tile([128, 8, S], f8, tag="xt")

            nc.vector.memset(v[:, :, :, HD : HD + 1], 1.0)

            with (
                tc.tile_pool(name="wvr", bufs=1) as wvrp,
                tc.tile_pool(name="qkw", bufs=2) as qkwp,
                tc.tile_pool(name="qts", bufs=2) as qtsp,
                tc.tile_pool(name="kts", bufs=2) as ktsp,
                tc.tile_pool(name="st", bufs=6) as stp,
                tc.tile_pool(name="stage", bufs=2) as stagep,
                tc.tile_pool(name="dens", bufs=1) as densp,
                tc.tile_pool(name="ps1", bufs=2, space="PSUM") as ps1,
                tc.tile_pool(name="sp", bufs=2, space="PSUM") as spp,
                tc.tile_pool(name="av", bufs=1, space="PSUM") as avp,
            ):
                wv_r = wvrp.tile([128, 8, D], f8, tag="wvr")
                for k in range(8):
                    nc.sync.dma_start(
                        out=wv_r[:, k, :], in_=wv_d[k * 128 : (k + 1) * 128, :]
                    )
                for r in range(8):
                    nc.sync.dma_start(
                        out=xt[:, r, 0:512], in_=xt_d[r * 128 : (r + 1) * 128, 0:512]
                    )
                for cc in range(1, 4):
                    for r in range(8):
                        nc.gpsimd.dma_start(
                            out=xt[:, r, cc * 512 : (cc + 1) * 512],
                            in_=xt_d[r * 128 : (r + 1) * 128, cc * 512 : (cc + 1) * 512],
                        )
                nc.sync.dma_start(out=bq_c[:], in_=bq_d[:])
                nc.sync.dma_start(out=bk_c[:], in_=bk_d[:])
                nc.sync.dma_start(out=sel2_c[:], in_=sel2_d[:])
                nc.gpsimd.dma_start(out=bv_c[:], in_=bcast_ap(bv_d))
                nc.gpsimd.dma_start(out=bo_c[:], in_=bcast_ap(bo_d))
                nc.gpsimd.dma_start(out=gamma_c[:], in_=bcast_ap(gamma_d))
                nc.gpsimd.dma_start(out=beta_c[:], in_=bcast_ap(beta_d))
                for tc_i in range(16):
                    for dg in range(2):
                        psv = ps1.tile([128, 512], f32, tag="ps", name="psv")
                        for a in range(4):
                            nc.tensor.matmul(
                                out=psv[:],
                                lhsT=xt[:, 2 * a : 2 * a + 2, tc_i * 128 : (tc_i + 1) * 128],
                                rhs=wv_r[:, 2 * a : 2 * a + 2, dg * 512 : (dg + 1) * 512],
                                start=(a == 0),
                                stop=(a == 3),
                                perf_mode=mybir.MatmulPerfMode.DoubleRow,
                            )
                        nc.vector.tensor_tensor(
                            out=v[:, tc_i, dg * 8 : (dg + 1) * 8, 0:HD],
                            in0=psv[:].rearrange("p (h d) -> p h d", d=HD),
                            in1=bv_c[:, dg * 512 : (dg + 1) * 512].rearrange(
                                "p (h d) -> p h d", d=HD
                            ),
                            op=ADD,
                        )

                def proj_block(m):
                    wq_m = qkwp.tile([128, 8, 128], f8, tag="qkw", name="wq_m")
                    nc.sync.dma_start(out=wq_m[:], in_=wq_d[:, m, :, :])
                    qt_m = qtsp.tile([128, SQ], bf16, tag="qts", name="qt_m")
                    for tg in range(2):
                        psq = ps1.tile([128, 512], f32, tag="ps", name="psq")
                        for a in range(4):
                            nc.tensor.matmul(
                                out=psq[:],
                                lhsT=wq_m[:, 2 * a : 2 * a + 2, :],
                                rhs=xt[:, 2 * a : 2 * a + 2, tg * 512 : (tg + 1) * 512],
                                start=(a == 0),
                                stop=(a == 3),
                                perf_mode=mybir.MatmulPerfMode.DoubleRow,
                            )
                        nc.vector.tensor_scalar(
                            out=qt_m[:, tg * 512 : (tg + 1) * 512],
                            in0=psq[:],
                            scalar1=bq_c[:, m : m + 1],
                            scalar2=None,
                            op0=ADD,
                        )
                    wk_m = qkwp.tile([128, 8, 128], f8, tag="qkw", name="wk_m")
                    nc.sync.dma_start(out=wk_m[:], in_=wk_d[:, m, :, :])
                    kt_m = ktsp.tile([128, S], bf16, tag="kts", name="kt_m")
                    for tg in range(4):
                        psk = ps1.tile([128, 512], f32, tag="ps", name="psk")
                        for a in range(4):
                            nc.tensor.matmul(
                                out=psk[:],
                                lhsT=wk_m[:, 2 * a : 2 * a + 2, :],
                                rhs=xt[:, 2 * a : 2 * a + 2, tg * 512 : (tg + 1) * 512],
                                start=(a == 0),
                                stop=(a == 3),
                                perf_mode=mybir.MatmulPerfMode.DoubleRow,
                            )
                        nc.vector.tensor_scalar(
                            out=kt_m[:, tg * 512 : (tg + 1) * 512],
                            in0=psk[:],
                            scalar1=bk_c[:, m : m + 1],
                            scalar2=None,
                            op0=ADD,
                        )
                    return qt_m, kt_m

                def qk_exp_block(m, qt_m, kt_m):
                    st_pair = []
                    for hh in range(2):
                        p0 = hh * 64
                        st_tiles = [
                            stp.tile([128, 4, SQ], f8, tag="st", name="st")
                            for _ in range(4)
                        ]
                        for kc in range(16):
                            sp = spp.tile([128, 1024], f32, tag="sp", name="sp")
                            for qh in range(2):
                                nc.tensor.matmul(
                                    out=sp[:, qh * 512 : (qh + 1) * 512],
                                    lhsT=kt_m[p0 : p0 + 64, kc * 128 : (kc + 1) * 128],
                                    rhs=qt_m[p0 : p0 + 64, qh * 512 : (qh + 1) * 512],
                                    start=True,
                                    stop=True,
                                )
                            nc.scalar.activation(
                                out=st_tiles[kc // 4][:, kc % 4, :],
                                in_=sp[:],
                                func=Exp,
                                scale=float(1.0 / np.sqrt(HD)),
                                bias=negone_c[:, 0:1],
                            )
                        st_pair.append(st_tiles)
                    return st_pair

                def av_block(m, st_pair):
                    den_m = densp.tile([2, SQ], f32, tag="den", name="den_m")
                    for hh in range(2):
                        h = 2 * m + hh
                        st_tiles = st_pair[hh]
                        av = avp.tile([128, 1024], f32, tag="av", name="av")
                        for qh in range(2):
                            for pi in range(8):
                                kc0 = 2 * pi
                                nc.tensor.matmul(
                                    out=av[0:65, qh * 512 : (qh + 1) * 512],
                                    lhsT=v[:, kc0 : kc0 + 2, h, :],
                                    rhs=st_tiles[kc0 // 4][
                                        :, kc0 % 4 : kc0 % 4 + 2,
                                        qh * 512 : (qh + 1) * 512,
                                    ],
                                    start=(pi == 0),
                                    stop=(pi == 7),
                                    perf_mode=mybir.MatmulPerfMode.DoubleRow,
                                )
                        stg_d = stagep.tile([65, SQ], f32, tag="stg_d", name="stg_d")
                        nc.vector.tensor_copy(out=stg_d[64:65, :], in_=av[64:65, :])
                        nc.sync.dma_start(out=den_m[hh : hh + 1, :], in_=stg_d[64:65, :])
                        if hh == 0:
                            nc.vector.tensor_copy(out=ot[0:64, m, :], in_=av[0:64, :])
                        else:
                            stg_o = stagep.tile([64, SQ], bf16, tag="stg_o", name="stg_o")
                            nc.vector.tensor_copy(out=stg_o[:, :], in_=av[0:64, :])
                            nc.sync.dma_start(out=ot[64:128, m, :], in_=stg_o[:, :])
                    rc_f = densp.tile([2, SQ], f32, tag="rcf", name="rc_f")
                    rc_b = densp.tile([2, SQ], bf16, tag="rcb", name="rc_b")
                    nc.vector.reciprocal_approx_fast(out=rc_f[:], in_=den_m[:])
                    nc.vector.tensor_copy(out=rc_b[:], in_=rc_f[:])
                    bc = avp.tile([128, 1024], f32, tag="av", name="bc")
                    for g in range(2):
                        nc.tensor.matmul(
                            out=bc[:, g * 512 : (g + 1) * 512],
                            lhsT=sel2_c[:],
                            rhs=rc_b[:, g * 512 : (g + 1) * 512],
                            start=True,
                            stop=True,
                        )
                    nc.vector.tensor_tensor(out=ot[:, m, :], in0=ot[:, m, :], in1=bc[:], op=MULT)

                prev = None
                for m in range(8):
                    qt_m, kt_m = proj_block(m)
                    st_pair = qk_exp_block(m, qt_m, kt_m)
                    if prev is not None:
                        av_block(prev[0], prev[1])
                    prev = (m, st_pair)
                av_block(prev[0], prev[1])

            with (
                tc.tile_pool(name="wo", bufs=8) as wop,
                tc.tile_pool(name="xqp", bufs=8) as xqp,
                tc.tile_pool(name="xqbo", bufs=8) as xqbop,
                tc.tile_pool(name="y", bufs=3) as yp,
                tc.tile_pool(name="y2", bufs=3) as y2p,
                tc.tile_pool(name="stats", bufs=4) as statp,
                tc.tile_pool(name="ps3", bufs=4, space="PSUM") as ps3,
            ):
                wo_tiles = []
                for k in range(8):
                    wt = wop.tile([128, D], bf16, tag="wo", name="wo_t")
                    nc.sync.dma_start(out=wt[:], in_=wo_d[k * 128 : (k + 1) * 128, :])
                    wo_tiles.append(wt)
                xq_tiles = []
                for t in range(8):
                    xq_t = xqp.tile([128, D], f32, tag="xq", name="xq_t")
                    nc.sync.dma_start(out=xq_t[:], in_=xq_d[t * 128 : (t + 1) * 128, :])
                    xq_tiles.append(xq_t)
                xqbo_tiles = []
                for t in range(8):
                    xqbo = xqbop.tile([128, D], f32, tag="xqbo", name="xqbo_t")
                    nc.gpsimd.tensor_tensor(
                        out=xqbo[:], in0=xq_tiles[t][:], in1=bo_c[:], op=ADD
                    )
                    xqbo_tiles.append(xqbo)
                for tg in range(2):
                    ps = [ps3.tile([128, 1024], f32, tag="ps", name="ps3") for _ in range(4)]
                    for k in range(8):
                        wt = wo_tiles[k]
                        for i in range(4):
                            t = tg * 4 + i
                            for g in range(2):
                                nc.tensor.matmul(
                                    out=ps[i][:, g * 512 : (g + 1) * 512],
                                    lhsT=ot[:, k, t * 128 : (t + 1) * 128],
                                    rhs=wt[:, g * 512 : (g + 1) * 512],
                                    start=(k == 0),
                                    stop=(k == 7),
                                )
                    for i in range(4):
                        t = tg * 4 + i
                        xqbo = xqbo_tiles[t]
                        y = yp.tile([128, D], f32, tag="y")
                        nc.vector.tensor_tensor(out=y[:], in0=ps[i][:], in1=xqbo[:], op=ADD)
                        stats = statp.tile([128, 2, 6], f32, tag="stats")
                        mv = statp.tile([128, 2], f32, tag="mv")
                        nc.vector.bn_stats(out=stats[:, 0, :], in_=y[:, 0:512])
                        nc.vector.bn_stats(out=stats[:, 1, :], in_=y[:, 512:1024])
                        nc.vector.bn_aggr(out=mv[:], in_=stats[:])
                        nc.scalar.activation(
                            out=mv[:, 1:2], in_=mv[:, 1:2], func=Sqrt, bias=eps_c[:, 0:1]
                        )
                        nc.vector.reciprocal(out=mv[:, 1:2], in_=mv[:, 1:2])
                        nc.vector.tensor_scalar(
                            out=y[:],
                            in0=y[:],
                            scalar1=mv[:, 0:1],
                            scalar2=mv[:, 1:2],
                            op0=SUB,
                            op1=MULT,
                        )
                        y2 = y2p.tile([128, D], f32, tag="y2")
                        nc.vector.tensor_tensor(out=y2[:], in0=y[:], in1=gamma_c[:], op=MULT)
                        nc.gpsimd.tensor_tensor(out=y2[:], in0=y2[:], in1=beta_c[:], op=ADD)
                        nc.sync.dma_start(out=out_d[t * 128 : (t + 1) * 128, :], in_=y2[:])

    nc.compile()
    return nc


def _get_nc():
    if "nc" not in _CACHE:
        _CACHE["nc"] = _build()
    return _CACHE["nc"]


def kernel(X, Wq, bq, Wk, bk, Wv, bv, Wo, bo, gamma, beta):
    if os.environ.get("BASS_TRACE"):
        _install_ntff_hook()
    import ml_dtypes

    from concourse.bass_utils import run_bass_kernel_spmd

    bfdt = ml_dtypes.bfloat16
    f8dt = ml_dtypes.float8_e4m3fn
    f32 = np.float32
    X = np.ascontiguousarray(np.asarray(X, dtype=f32))
    wqt = np.ascontiguousarray(
        np.asarray(Wq, f32).T.astype(f8dt).reshape(8, 128, 8, 128).transpose(1, 2, 0, 3)
    )
    wkt = np.ascontiguousarray(
        np.asarray(Wk, f32).T.astype(f8dt).reshape(8, 128, 8, 128).transpose(1, 2, 0, 3)
    )
    wvt = np.ascontiguousarray(np.asarray(Wv, f32).T.astype(f8dt))
    wot = np.ascontiguousarray(np.asarray(Wo, f32).T.astype(bfdt))
    bqt = np.ascontiguousarray(np.asarray(bq, f32).reshape(8, 128).T)
    bkt = np.ascontiguousarray(np.asarray(bk, f32).reshape(8, 128).T)
    bv_ = np.ascontiguousarray(np.asarray(bv, f32))
    bo_ = np.ascontiguousarray(np.asarray(bo, f32))
    gamma_ = np.ascontiguousarray(np.asarray(gamma, f32))
    beta_ = np.ascontiguousarray(np.asarray(beta, f32))
    sel2 = np.zeros((2, 128), f32)
    sel2[0, 0:64] = 1.0
    sel2[1, 64:128] = 1.0
    sel2 = sel2.astype(bfdt)

    in_maps = []
    for c in range(NCORES):
        b, half = c // 2, c % 2
        Xb = X[b]
        q_rows = Xb[half * SQ : (half + 1) * SQ]
        o_rows = Xb[(1 - half) * SQ : (2 - half) * SQ]
        xt = np.ascontiguousarray(np.concatenate([q_rows, o_rows], axis=0).T.astype(f8dt))
        in_maps.append(
            {
                "xt": xt,
                "xq": np.ascontiguousarray(q_rows),
                "wqt": wqt,
                "wkt": wkt,
                "wvt": wvt,
                "wot": wot,
                "bqt": bqt,
                "bkt": bkt,
                "bv": bv_,
                "bo": bo_,
                "gamma": gamma_,
                "beta": beta_,
                "sel2": sel2,
            }
        )

    nc = _get_nc()
    res = run_bass_kernel_spmd(nc, in_maps, core_ids=list(range(NCORES)))
    if res.exec_time_ns is not None:
        print(f"HW exec time: {res.exec_time_ns} ns")

    out = np.empty((B, S, D), np.float32)
    for c in range(NCORES):
        b, half = c // 2, c % 2
        out[b, half * SQ : (half + 1) * SQ] = res.results[c]["out"]
    return out


# revision 17
# speedup vs baseline: 1.1949x; 1.0611x over previous
"""Baseline structure + targeted fixes only:
- softmax reciprocal via single-op DVE approximation (was 6.4us InstReciprocal
  on the critical path, causing a per-pair PE stall + HAM re-throttle)
- wq/wk pre-tiled DRAM layout (2KB DMA descriptors instead of 256B)
- xt DMA split into 512-col chunks (earlier first matmul)
"""

import os
import sys

sys.path.insert(0, "/opt/trn_rl_repo")

import numpy as np

B, S, D, H = 4, 2048, 1024, 16
HD = D // H
SQ = S // 2
NCORES = 8
EPS = 1e-12

_CACHE = {}


def _install_ntff_hook():
    import contextlib
    import ctypes
    import types

    so_path = "/opt/axon/libaxon_pjrt.so"
    if "antenv.axon_hooks" in sys.modules:
        return
    try:
        lib = ctypes.CDLL(so_path)
    except OSError:
        return
    if not hasattr(lib, "axon_start_nrt_profile"):
        return
    lib.axon_start_nrt_profile.argtypes = [ctypes.POINTER(ctypes.c_int64), ctypes.c_size_t]
    lib.axon_start_nrt_profile.restype = ctypes.c_int64
    lib.axon_stop_nrt_profile.argtypes = [ctypes.c_char_p]
    lib.axon_stop_nrt_profile.restype = ctypes.c_int64

    @contextlib.contextmanager
    def _hook(output_dir, device_ids):
        import jax

        jax.devices()
        if device_ids:
            ids = (ctypes.c_int64 * len(device_ids))(*device_ids)
            rc = lib.axon_start_nrt_profile(ids, len(device_ids))
        else:
            rc = lib.axon_start_nrt_profile(None, 0)
        if rc != 0:
            raise RuntimeError(f"axon_start_nrt_profile rc={rc}")
        try:
            yield
        finally:
            n = lib.axon_stop_nrt_profile(str(output_dir).encode())
            if n < 0:
                raise RuntimeError(f"axon_stop_nrt_profile rc={n}")

    m = types.ModuleType("antenv.axon_hooks")
    m.get_axon_ntff_profile_hook = lambda: _hook
    m.set_axon_ntff_profile_hook = lambda h: None
    sys.modules["antenv.axon_hooks"] = m


def _build():
    import concourse.bass as bass
    import concourse.tile as tile
    from concourse import bacc, mybir

    f32 = mybir.dt.float32
    bf16 = mybir.dt.bfloat16
    f8 = mybir.dt.float8e4
    ADD = mybir.AluOpType.add
    MULT = mybir.AluOpType.mult
    SUB = mybir.AluOpType.subtract
    Exp = mybir.ActivationFunctionType.Exp
    Sqrt = mybir.ActivationFunctionType.Sqrt

    nc = bacc.Bacc("TRN2")

    xt_d = nc.dram_tensor("xt", [D, S], f8, kind="ExternalInput")
    xq_d = nc.dram_tensor("xq", [SQ, D], f32, kind="ExternalInput")
    wq_d = nc.dram_tensor("wqt", [128, 8, 8, 128], f8, kind="ExternalInput")
    wk_d = nc.dram_tensor("wkt", [128, 8, 8, 128], f8, kind="ExternalInput")
    wv_d = nc.dram_tensor("wvt", [D, D], f8, kind="ExternalInput")
    wo_d = nc.dram_tensor("wot", [D, D], bf16, kind="ExternalInput")
    bq_d = nc.dram_tensor("bqt", [128, 8], f32, kind="ExternalInput")
    bk_d = nc.dram_tensor("bkt", [128, 8], f32, kind="ExternalInput")
    bv_d = nc.dram_tensor("bv", [D], f32, kind="ExternalInput")
    bo_d = nc.dram_tensor("bo", [D], f32, kind="ExternalInput")
    gamma_d = nc.dram_tensor("gamma", [D], f32, kind="ExternalInput")
    beta_d = nc.dram_tensor("beta", [D], f32, kind="ExternalInput")
    sel2_d = nc.dram_tensor("sel2", [2, 128], bf16, kind="ExternalInput")
    out_d = nc.dram_tensor("out", [SQ, D], f32, kind="ExternalOutput")

    def bcast_ap(handle):
        ap = handle[:]
        return bass.AP(tensor=ap.tensor, offset=ap.offset, ap=[[0, 128], ap.ap[0]])

    with tile.TileContext(nc) as tc:
        with (
            tc.tile_pool(name="const", bufs=1) as constp,
            tc.tile_pool(name="v", bufs=1) as vp,
            tc.tile_pool(name="ot", bufs=1) as otp,
            tc.tile_pool(name="xt", bufs=1) as xtp,
        ):
            bq_c = constp.tile([128, 8], f32, tag="bq")
            bk_c = constp.tile([128, 8], f32, tag="bk")
            bv_c = constp.tile([128, D], f32, tag="bv")
            bo_c = constp.tile([128, D], f32, tag="bo")
            gamma_c = constp.tile([128, D], f32, tag="gamma")
            beta_c = constp.tile([128, D], f32, tag="beta")
            eps_c = constp.tile([128, 1], f32, tag="eps")
            negone_c = constp.tile([128, 1], f32, tag="negone")
            sel2_c = constp.tile([2, 128], bf16, tag="sel2")
            nc.vector.memset(eps_c[:], EPS)
            nc.vector.memset(negone_c[:], -3.5)

            v = vp.tile([128, 16, H, HD + 1], f8, tag="v")
            ot = otp.tile([128, 8, SQ], bf16, tag="ot")
            xt = xtp.tile([128, 8, S], f8, tag="xt")

            nc.vector.memset(v[:, :, :, HD : HD + 1], 1.0)

            with (
                tc.tile_pool(name="wvr", bufs=1) as wvrp,
                tc.tile_pool(name="qkw", bufs=2) as qkwp,
                tc.tile_pool(name="qts", bufs=2) as qtsp,
                tc.tile_pool(name="kts", bufs=2) as ktsp,
                tc.tile_pool(name="st", bufs=6) as stp,
                tc.tile_pool(name="stage", bufs=2) as stagep,
                tc.tile_pool(name="dens", bufs=1) as densp,
                tc.tile_pool(name="ps1", bufs=2, space="PSUM") as ps1,
                tc.tile_pool(name="sp", bufs=2, space="PSUM") as spp,
                tc.tile_pool(name="av", bufs=2, space="PSUM") as avp,
            ):
                wv_r = wvrp.tile([128, 8, D], f8, tag="wvr")
                for k in range(8):
                    nc.sync.dma_start(
                        out=wv_r[:, k, :], in_=wv_d[k * 128 : (k + 1) * 128, :]
                    )
                for r in range(8):
                    nc.sync.dma_start(
                        out=xt[:, r, 0:512], in_=xt_d[r * 128 : (r + 1) * 128, 0:512]
                    )
                for cc in range(1, 4):
                    for r in range(8):
                        nc.gpsimd.dma_start(
                            out=xt[:, r, cc * 512 : (cc + 1) * 512],
                            in_=xt_d[r * 128 : (r + 1) * 128, cc * 512 : (cc + 1) * 512],
                        )
                nc.sync.dma_start(out=bq_c[:], in_=bq_d[:])
                nc.sync.dma_start(out=bk_c[:], in_=bk_d[:])
                nc.sync.dma_start(out=sel2_c[:], in_=sel2_d[:])
                nc.gpsimd.dma_start(out=bv_c[:], in_=bcast_ap(bv_d))
                nc.gpsimd.dma_start(out=bo_c[:], in_=bcast_ap(bo_d))
                nc.gpsimd.dma_start(out=gamma_c[:], in_=bcast_ap(gamma_d))
                nc.gpsimd.dma_start(out=beta_c[:], in_=bcast_ap(beta_d))
                for tc_i in range(16):
                    for dg in range(2):
                        psv = ps1.tile([128, 512], f32, tag="ps", name="psv")
                        for a in range(4):
                            nc.tensor.matmul(
                                out=psv[:],
                                lhsT=xt[:, 2 * a : 2 * a + 2, tc_i * 128 : (tc_i + 1) * 128],
                                rhs=wv_r[:, 2 * a : 2 * a + 2, dg * 512 : (dg + 1) * 512],
                                start=(a == 0),
                                stop=(a == 3),
                                perf_mode=mybir.MatmulPerfMode.DoubleRow,
                            )
                        nc.vector.tensor_tensor(
                            out=v[:, tc_i, dg * 8 : (dg + 1) * 8, 0:HD],
                            in0=psv[:].rearrange("p (h d) -> p h d", d=HD),
                            in1=bv_c[:, dg * 512 : (dg + 1) * 512].rearrange(
                                "p (h d) -> p h d", d=HD
                            ),
                            op=ADD,
                        )

                def proj_block(m):
                    wq_m = qkwp.tile([128, 8, 128], f8, tag="qkw", name="wq_m")
                    nc.sync.dma_start(out=wq_m[:], in_=wq_d[:, m, :, :])
                    qt_m = qtsp.tile([128, SQ], bf16, tag="qts", name="qt_m")
                    for tg in range(2):
                        psq = ps1.tile([128, 512], f32, tag="ps", name="psq")
                        for a in range(4):
                            nc.tensor.matmul(
                                out=psq[:],
                                lhsT=wq_m[:, 2 * a : 2 * a + 2, :],
                                rhs=xt[:, 2 * a : 2 * a + 2, tg * 512 : (tg + 1) * 512],
                                start=(a == 0),
                                stop=(a == 3),
                                perf_mode=mybir.MatmulPerfMode.DoubleRow,
                            )
                        nc.vector.tensor_scalar(
                            out=qt_m[:, tg * 512 : (tg + 1) * 512],
                            in0=psq[:],
                            scalar1=bq_c[:, m : m + 1],
                            scalar2=None,
                            op0=ADD,
                        )
                    wk_m = qkwp.tile([128, 8, 128], f8, tag="qkw", name="wk_m")
                    nc.sync.dma_start(out=wk_m[:], in_=wk_d[:, m, :, :])
                    kt_m = ktsp.tile([128, S], bf16, tag="kts", name="kt_m")
                    for tg in range(4):
                        psk = ps1.tile([128, 512], f32, tag="ps", name="psk")
                        for a in range(4):
                            nc.tensor.matmul(
                                out=psk[:],
                                lhsT=wk_m[:, 2 * a : 2 * a + 2, :],
                                rhs=xt[:, 2 * a : 2 * a + 2, tg * 512 : (tg + 1) * 512],
                                start=(a == 0),
                                stop=(a == 3),
                                perf_mode=mybir.MatmulPerfMode.DoubleRow,
                            )
                        nc.vector.tensor_scalar(
                            out=kt_m[:, tg * 512 : (tg + 1) * 512],
                            in0=psk[:],
                            scalar1=bk_c[:, m : m + 1],
                            scalar2=None,
                            op0=ADD,
                        )
                    return qt_m, kt_m

                def qk_exp_block(m, qt_m, kt_m):
                    st_pair = []
                    for hh in range(2):
                        p0 = hh * 64
                        st_tiles = [
                            stp.tile([128, 4, SQ], f8, tag="st", name="st")
                            for _ in range(4)
                        ]
                        for kc in range(16):
                            sp = spp.tile([128, 1024], f32, tag="sp", name="sp")
                            for qh in range(2):
                                nc.tensor.matmul(
                                    out=sp[:, qh * 512 : (qh + 1) * 512],
                                    lhsT=kt_m[p0 : p0 + 64, kc * 128 : (kc + 1) * 128],
                                    rhs=qt_m[p0 : p0 + 64, qh * 512 : (qh + 1) * 512],
                                    start=True,
                                    stop=True,
                                )
                            nc.scalar.activation(
                                out=st_tiles[kc // 4][:, kc % 4, :],
                                in_=sp[:],
                                func=Exp,
                                scale=float(1.0 / np.sqrt(HD)),
                                bias=negone_c[:, 0:1],
                            )
                        st_pair.append(st_tiles)
                    return st_pair

                def av_block(m, st_pair):
                    den_m = densp.tile([2, SQ], f32, tag="den", name="den_m")
                    for hh in range(2):
                        h = 2 * m + hh
                        st_tiles = st_pair[hh]
                        for qh in range(2):
                            av = avp.tile([128, 512], f32, tag="av", name="av")
                            for pi in range(8):
                                kc0 = 2 * pi
                                nc.tensor.matmul(
                                    out=av[0:65, :],
                                    lhsT=v[:, kc0 : kc0 + 2, h, :],
                                    rhs=st_tiles[kc0 // 4][
                                        :, kc0 % 4 : kc0 % 4 + 2,
                                        qh * 512 : (qh + 1) * 512,
                                    ],
                                    start=(pi == 0),
                                    stop=(pi == 7),
                                    perf_mode=mybir.MatmulPerfMode.DoubleRow,
                                )
                            stg_d = stagep.tile([65, 512], f32, tag="stg_d", name="stg_d")
                            nc.vector.tensor_copy(out=stg_d[64:65, :], in_=av[64:65, :])
                            nc.sync.dma_start(
                                out=den_m[hh : hh + 1, qh * 512 : (qh + 1) * 512],
                                in_=stg_d[64:65, :],
                            )
                            if hh == 0:
                                nc.vector.tensor_copy(
                                    out=ot[0:64, m, qh * 512 : (qh + 1) * 512],
                                    in_=av[0:64, :],
                                )
                            else:
                                stg_o = stagep.tile(
                                    [64, 512], bf16, tag="stg_o", name="stg_o"
                                )
                                nc.vector.tensor_copy(out=stg_o[:, :], in_=av[0:64, :])
                                nc.sync.dma_start(
                                    out=ot[64:128, m, qh * 512 : (qh + 1) * 512],
                                    in_=stg_o[:, :],
                                )
                    rc_f = densp.tile([2, SQ], f32, tag="rcf", name="rc_f")
                    rc_b = densp.tile([2, SQ], bf16, tag="rcb", name="rc_b")
                    nc.vector.reciprocal_approx_fast(out=rc_f[:], in_=den_m[:])
                    nc.vector.tensor_copy(out=rc_b[:], in_=rc_f[:])
                    for g in range(2):
                        bc = avp.tile([128, 512], f32, tag="av", name="bc")
                        nc.tensor.matmul(
                            out=bc[:],
                            lhsT=sel2_c[:],
                            rhs=rc_b[:, g * 512 : (g + 1) * 512],
                            start=True,
                            stop=True,
                        )
                        nc.vector.tensor_tensor(
                            out=ot[:, m, g * 512 : (g + 1) * 512],
                            in0=ot[:, m, g * 512 : (g + 1) * 512],
                            in1=bc[:],
                            op=MULT,
                        )

                prev = None
                for m in range(8):
                    qt_m, kt_m = proj_block(m)
                    st_pair = qk_exp_block(m, qt_m, kt_m)
                    if prev is not None:
                        av_block(prev[0], prev[1])
                    prev = (m, st_pair)
                av_block(prev[0], prev[1])

            with (
                tc.tile_pool(name="wo", bufs=8) as wop,
                tc.tile_pool(name="xqp", bufs=8) as xqp,
                tc.tile_pool(name="xqbo", bufs=8) as xqbop,
                tc.tile_pool(name="y", bufs=3) as yp,
                tc.tile_pool(name="y2", bufs=3) as y2p,
                tc.tile_pool(name="stats", bufs=4) as statp,
                tc.tile_pool(name="ps3", bufs=4, space="PSUM") as ps3,
            ):
                wo_tiles = []
                for k in range(8):
                    wt = wop.tile([128, D], bf16, tag="wo", name="wo_t")
                    nc.sync.dma_start(out=wt[:], in_=wo_d[k * 128 : (k + 1) * 128, :])
                    wo_tiles.append(wt)
                xq_tiles = []
                for t in range(8):
                    xq_t = xqp.tile([128, D], f32, tag="xq", name="xq_t")
                    nc.sync.dma_start(out=xq_t[:], in_=xq_d[t * 128 : (t + 1) * 128, :])
                    xq_tiles.append(xq_t)
                xqbo_tiles = []
                for t in range(8):
                    xqbo = xqbop.tile([128, D], f32, tag="xqbo", name="xqbo_t")
                    nc.gpsimd.tensor_tensor(
                        out=xqbo[:], in0=xq_tiles[t][:], in1=bo_c[:], op=ADD
                    )
                    xqbo_tiles.append(xqbo)
                for tg in range(2):
                    ps = [ps3.tile([128, 1024], f32, tag="ps", name="ps3") for _ in range(4)]
                    for k in range(8):
                        wt = wo_tiles[k]
                        for i in range(4):
                            t = tg * 4 + i
                            for g in range(2):
                                nc.tensor.matmul(
                                    out=ps[i][:, g * 512 : (g + 1) * 512],
                                    lhsT=ot[:, k, t * 128 : (t + 1) * 128],
                                    rhs=wt[:, g * 512 : (g + 1) * 512],
                                    start=(k == 0),
                                    stop=(k == 7),
                                )
                    for i in range(4):
                        t = tg * 4 + i
                        xqbo = xqbo_tiles[t]
                        y = yp.tile([128, D], f32, tag="y")
                        nc.vector.tensor_tensor(out=y[:], in0=ps[i][:], in1=xqbo[:], op=ADD)
                        stats = statp.tile([128, 2, 6], f32, tag="stats")
                        mv = statp.tile([128, 2], f32, tag="mv")
                        nc.vector.bn_stats(out=stats[:, 0, :], in_=y[:, 0:512])
                        nc.vector.bn_stats(out=stats[:, 1, :], in_=y[:, 512:1024])
                        nc.vector.bn_aggr(out=mv[:], in_=stats[:])
                        nc.scalar.activation(
                            out=mv[:, 1:2], in_=mv[:, 1:2], func=Sqrt, bias=eps_c[:, 0:1]
                        )
                        nc.vector.reciprocal(out=mv[:, 1:2], in_=mv[:, 1:2])
                        nc.vector.tensor_scalar(
                            out=y[:],
                            in0=y[:],
                            scalar1=mv[:, 0:1],
                            scalar2=mv[:, 1:2],
                            op0=SUB,
                            op1=MULT,
                        )
                        y2 = y2p.tile([128, D], f32, tag="y2")
                        nc.vector.tensor_tensor(out=y2[:], in0=y[:], in1=gamma_c[:], op=MULT)
                        nc.gpsimd.tensor_tensor(out=y2[:], in0=y2[:], in1=beta_c[:], op=ADD)
                        nc.sync.dma_start(out=out_d[t * 128 : (t + 1) * 128, :], in_=y2[:])

    nc.compile()
    return nc


def _get_nc():
    if "nc" not in _CACHE:
        _CACHE["nc"] = _build()
    return _CACHE["nc"]


def kernel(X, Wq, bq, Wk, bk, Wv, bv, Wo, bo, gamma, beta):
    if os.environ.get("BASS_TRACE"):
        _install_ntff_hook()
    import ml_dtypes

    from concourse.bass_utils import run_bass_kernel_spmd

    bfdt = ml_dtypes.bfloat16
    f8dt = ml_dtypes.float8_e4m3fn
    f32 = np.float32
    X = np.ascontiguousarray(np.asarray(X, dtype=f32))
    wqt = np.ascontiguousarray(
        np.asarray(Wq, f32).T.astype(f8dt).reshape(8, 128, 8, 128).transpose(1, 2, 0, 3)
    )
    wkt = np.ascontiguousarray(
        np.asarray(Wk, f32).T.astype(f8dt).reshape(8, 128, 8, 128).transpose(1, 2, 0, 3)
    )
    wvt = np.ascontiguousarray(np.asarray(Wv, f32).T.astype(f8dt))
    wot = np.ascontiguousarray(np.asarray(Wo, f32).T.astype(bfdt))
    bqt = np.ascontiguousarray(np.asarray(bq, f32).reshape(8, 128).T)
    bkt = np.ascontiguousarray(np.asarray(bk, f32).reshape(8, 128).T)
    bv_ = np.ascontiguousarray(np.asarray(bv, f32))
    bo_ = np.ascontiguousarray(np.asarray(bo, f32))
    gamma_ = np.ascontiguousarray(np.asarray(gamma, f32))
    beta_ = np.ascontiguousarray(np.asarray(beta, f32))
    sel2 = np.zeros((2, 128), f32)
    sel2[0, 0:64] = 1.0
    sel2[1, 64:128] = 1.0
    sel2 = sel2.astype(bfdt)

    in_maps = []
    for c in range(NCORES):
        b, half = c // 2, c % 2
        Xb = X[b]
        q_rows = Xb[half * SQ : (half + 1) * SQ]
        o_rows = Xb[(1 - half) * SQ : (2 - half) * SQ]
        xt = np.ascontiguousarray(np.concatenate([q_rows, o_rows], axis=0).T.astype(f8dt))
        in_maps.append(
            {
                "xt": xt,
                "xq": np.ascontiguousarray(q_rows),
                "wqt": wqt,
                "wkt": wkt,
                "wvt": wvt,
                "wot": wot,
                "bqt": bqt,
                "bkt": bkt,
                "bv": bv_,
                "bo": bo_,
                "gamma": gamma_,
                "beta": beta_,
                "sel2": sel2,
            }
        )

    nc = _get_nc()
    res = run_bass_kernel_spmd(nc, in_maps, core_ids=list(range(NCORES)))
    if res.exec_time_ns is not None:
        print(f"HW exec time: {res.exec_time_ns} ns")

    out = np.empty((B, S, D), np.float32)
    for c in range(NCORES):
        b, half = c // 2, c % 2
        out[b, half * SQ : (half + 1) * SQ] = res.results[c]["out"]
    return out
